# revision 2
# baseline (speedup 1.0000x reference)
"""
Trainium2 Bass kernel v2 for nn_CVXPolicy_DoubleIntegrator
(131072 x 192 -> 131072 x 96), 8 cores data-parallel.

Math:  p = MLP(concat([t,z])) (tanh x2, hidden 100); q = -(p@S + b3@S);
       u* = q/(1+s), s(1+s)^2 = ||q||^2 (closed-form Cardano).

v2 design (per core, 32 blocks of 512 rows, transposed layout):
  - fp16 wire dtypes everywhere (in z/t, weights, h1/h2, out u) -> half DMA.
    End-to-end precision ~1.1e-3 scale-rel (verified vs fp32 reference).
  - Dense input packing: xa [128,512]/block (z feats 0:128), xb2 pair-packs
    two blocks' 64-feature tails on partitions 0:64/64:128, t-rows in a
    [128,512] table consumed by one-hot(x)w1t stationaries (K=32, aligned).
  - PE tile concurrency: L1b pair (rg0-1 || rg2-3), t-mm pair (strip s vs
    s+1), L3 (M=96, cg0-2) || r-reduce (one-hot -> rps partitions 96:128,
    cg3), cb broadcast (stationary at partitions 96:128, rg3).
  - Solve runs on [32,512] at partitions 96:128: ACT chain (Square/Ln/Exp
    grouped to limit act-table switches) + DVE reciprocals; c in fp16.
  - ph3 (cb matmul + ut = q*c + store) of iteration i interleaves into
    iteration i+1's ticks; REPEAT>1 gives steady-state pipelining.
"""

import os
import sys
import numpy as np
from contextlib import ExitStack

for _p in ("/opt/trn_rl_repo", "/root/.axon_site/_ro/trn_rl_repo"):
    if _p not in sys.path and os.path.isdir(_p):
        sys.path.append(_p)

B_TOTAL = 131072
N_CORES = 8
BC = B_TOTAL // N_CORES          # 16384 rows per core
SD = 192
CD = 96
HID = 100
BLKN = 512
NBLK = BC // BLKN                # 32
SUPER = 4                        # blocks per DMA superblock
NSUP = NBLK // SUPER             # 8
SUPN = SUPER * BLKN              # 2048
NPAIR = NBLK // 2                # 16
REPEAT = int(os.environ.get("K_REPEAT", "1"))
C16 = 593                        # w1a|w1b|w1t|w2|w3|ones-col|ones96

_PROG_CACHE = {}


def _scatter_matrix():
    n = 32
    u_idx = np.arange(n)
    p_idx = np.concatenate([3 * np.arange(1, n + 1), 4 * np.arange(1, n + 1),
                            5 * np.arange(1, n + 1)])
    uu_idx = np.concatenate([u_idx, 2 * u_idx, 3 * u_idx])
    S = np.zeros((SD, CD), np.float64)
    for pi, ui in zip(p_idx, uu_idx):
        S[pi, ui] += 1.0
    return S


def _tsp_row(b):
    return 32 * (b % 3) + (b // 3)   # strips 0..2 (q3 unusable), rows 0..10


def _build_program():
    import concourse.mybir as mybir
    from concourse import bacc
    from concourse.tile import TileContext

    f32 = mybir.dt.float32
    f16 = mybir.dt.float16
    FT = mybir.ActivationFunctionType
    ALU = mybir.AluOpType
    fnp = np.float32

    nc = bacc.Bacc("TRN2", target_bir_lowering=False, debug=False,
                   num_devices=N_CORES)

    def din(name, shape, dt):
        return nc.dram_tensor(name, shape, dt, kind="ExternalInput").ap()

    xa_d = din("xa", [NSUP, 128, SUPN], f16)
    xb2_d = din("xb2", [NSUP, 128, SUPN // 2], f16)
    tsp_d = din("tsp", [128, BLKN], f16)
    cpk16_d = din("cpk16", [128, C16], f16)
    cpk32_d = din("cpk32", [128, 8], f32)
    rsel_d = din("rsel", [96, 32 * 32], f16)
    sels_d = din("sels", [32, 32 * 96], f16)
    tsel_d = din("tsel", [128, 11 * 100], f16)
    uT_d = nc.dram_tensor("uT", [NSUP, CD, SUPN], f16,
                          kind="ExternalOutput").ap()

    with TileContext(nc) as tc, ExitStack() as ctx:
        ctx.enter_context(nc.allow_low_precision(
            reason="fp16 wire dtypes; accumulation stays fp32 in PSUM"))
        consts = ctx.enter_context(tc.tile_pool(name="consts", bufs=1))
        cpk16 = consts.tile([128, C16], f16)
        cpk32 = consts.tile([128, 8], f32)
        rsel = consts.tile([96, 32 * 32], f16)
        sels = consts.tile([32, 32 * 96], f16)
        tsel = consts.tile([128, 11 * 100], f16)
        nc.scalar.dma_start(out=cpk16[:], in_=cpk16_d[:])
        nc.scalar.dma_start(out=cpk32[:], in_=cpk32_d[:])
        nc.scalar.dma_start(out=rsel[:], in_=rsel_d[:])
        nc.scalar.dma_start(out=sels[:], in_=sels_d[:])
        nc.scalar.dma_start(out=tsel[:], in_=tsel_d[:])
        w1a = cpk16[0:128, 0:100]
        w1b = cpk16[:, 100:200]          # rows 0:64 and 64:128 both hold tails
        w2 = cpk16[0:100, 300:400]
        w3 = cpk16[0:100, 400:496]
        b1 = cpk32[0:100, 0:1]
        b2 = cpk32[0:100, 1:2]
        b3 = cpk32[0:96, 3:4]
        c227 = cpk32[0:32, 4:5]
        c127 = cpk32[0:32, 5:6]

        qsb = consts.tile([CD, BC], f32)         # persistent q

        xpool = ctx.enter_context(tc.tile_pool(name="x", bufs=3))
        tpool = ctx.enter_context(tc.tile_pool(name="tsp", bufs=2))
        hpool = ctx.enter_context(tc.tile_pool(name="h", bufs=6))
        qqpool = ctx.enter_context(tc.tile_pool(name="qq", bufs=6))
        opool = ctx.enter_context(tc.tile_pool(name="o", bufs=3))
        ppool = ctx.enter_context(tc.tile_pool(name="p2", bufs=2))
        mm1 = ctx.enter_context(tc.tile_pool(name="mm1", bufs=2, space="PSUM"))
        mm2 = ctx.enter_context(tc.tile_pool(name="mm2", bufs=2, space="PSUM"))
        mmq = ctx.enter_context(tc.tile_pool(name="mmq", bufs=2, space="PSUM"))
        cps = ctx.enter_context(tc.tile_pool(name="cps", bufs=1, space="PSUM"))
        rpsp = ctx.enter_context(tc.tile_pool(name="rps", bufs=1,
                                              space="PSUM"))

        NSPL = 2
        H = BLKN // NSPL

        def phase2(rps, cstack):
            """rps [128,512] PSUM (rows 96:128 = r). Writes c fp16 into
            cstack rows 96:128. Returns list of emission thunks; ACT ops
            grouped to bound act-table switches."""
            P = slice(0, 32)
            halves = [slice(i * H, (i + 1) * H) for i in range(NSPL)]
            t_sq = ppool.tile([32, BLKN], f32, tag="p2a", name="p2sq")
            t_sd = ppool.tile([32, BLKN], f32, tag="p2b", name="p2sd")
            t_u3 = ppool.tile([32, BLKN], f32, tag="p2c", name="p2u3")
            t_u = ppool.tile([32, BLKN], f32, tag="p2d", name="p2u")
            t_ru = ppool.tile([32, BLKN], f32, tag="p2e", name="p2ru")
            t_y = ppool.tile([32, BLKN], f32, tag="p2f", name="p2y")

            def th(fn):
                return [lambda h=h: fn(h) for h in halves]

            jobs = []
            # sq = (r + 2/27)^2   [ACT, Square is in the tanh set]
            jobs += th(lambda h: nc.scalar.activation(
                out=t_sq[P, h], in_=rps[P, h], func=FT.Square,
                bias=c227))
            # d' = max(sq - 4/729, eps)
            jobs += th(lambda h: nc.vector.tensor_scalar(
                out=t_sq[P, h], in0=t_sq[P, h],
                scalar1=float(fnp(-4.0 / 729.0)), scalar2=float(fnp(1e-30)),
                op0=ALU.add, op1=ALU.max))
            # sd = exp(0.5*ln(d'))  [ACT: Ln then Exp — grouped below]
            jobs += th(lambda h: nc.scalar.activation(
                out=t_sd[P, h], in_=t_sq[P, h], func=FT.Ln))
            jobs += th(lambda h: nc.scalar.activation(
                out=t_sd[P, h], in_=t_sd[P, h], func=FT.Exp, scale=0.5))
            # u32 = r + sd
            jobs += th(lambda h: nc.vector.tensor_tensor(
                out=t_u3[P, h], in0=rps[P, h], in1=t_sd[P, h], op=ALU.add))
            # u = exp(ln(0.5*u32 + 1/27)/3)
            jobs += th(lambda h: nc.scalar.activation(
                out=t_u[P, h], in_=t_u3[P, h], func=FT.Ln, scale=0.5,
                bias=c127))
            jobs += th(lambda h: nc.scalar.activation(
                out=t_u[P, h], in_=t_u[P, h], func=FT.Exp,
                scale=float(fnp(1.0 / 3.0))))
            # c = 1/(u + 1/(9u) + 1/3)
            jobs += th(lambda h: nc.vector.reciprocal(
                out=t_ru[P, h], in_=t_u[P, h]))
            jobs += th(lambda h: nc.vector.tensor_scalar(
                out=t_ru[P, h], in0=t_ru[P, h],
                scalar1=float(fnp(1.0 / 9.0)), scalar2=float(fnp(1.0 / 3.0)),
                op0=ALU.mult, op1=ALU.add))
            jobs += th(lambda h: nc.vector.tensor_tensor(
                out=t_y[P, h], in0=t_u[P, h], in1=t_ru[P, h], op=ALU.add))
            jobs += th(lambda h: nc.vector.reciprocal(
                out=cstack[P, h], in_=t_y[P, h]))
            return jobs

        # ---------------- pipelined emission ----------------
        STG_T1, STG_L2, STG_T2, STG_L3, STG_QM, STG_QS = 1, 2, 3, 4, 5, 6
        R_LAG = 7
        OFF3 = 4
        NTICK = NPAIR + R_LAG + 2     # 25

        st_x, st_h1p, st_h1, st_h2p, st_h2 = {}, {}, {}, {}, {}
        st_qp, st_qq, st_cb, st_ut = {}, {}, {}, {}

        def l1_emit(j, rps_unused):
            su, so = divmod(j, SUPER)
            if so == 0 and (su % 1 == 0):
                for psu in ([su, su + 1] if j == 0 else [su + 1]):
                    if psu >= NSUP or psu in st_x:
                        continue
                    xa = xpool.tile([128, SUPN], f16, tag="xa", name="xa")
                    nc.sync.dma_start(out=xa[:], in_=xa_d[psu])
                    xb = xpool.tile([128, SUPN // 2], f16, tag="xb", name="xb")
                    nc.scalar.dma_start(out=xb[:], in_=xb2_d[psu])
                    st_x[psu] = (xa, xb)
            xa, xb = st_x[su]
            ns = slice(so * BLKN, (so + 1) * BLKN)
            h1p = mm1.tile([HID, BLKN], f32, tag="h1p", name="h1p")
            nc.tensor.matmul(h1p[:], w1a, xa[:, ns], start=True, stop=False)
            st_h1p[j] = (h1p, xb)

        def l1_tail(j):
            h1p, xb = st_h1p[j]
            su, so = divmod(j, SUPER)
            m, k = so % 2, so // 2
            nc.tensor.matmul(h1p[:], w1b[64 * m:64 * m + 64, :],
                             xb[64 * m:64 * m + 64, 512 * k:512 * k + 512],
                             start=False, stop=False)

        def t_emit(j, tsp):
            h1p, _ = st_h1p[j]
            s, v = j % 3, j // 3
            nc.tensor.matmul(h1p[:], tsel[32 * s:32 * s + 32,
                                          100 * v:100 * v + 100],
                             tsp[32 * s:32 * s + 32, :],
                             start=False, stop=True)

        def tanh1_emit(j):
            h1p, _ = st_h1p.pop(j)
            h1 = hpool.tile([HID, BLKN], f16, tag="h1", name="h1")
            nc.scalar.activation(out=h1[:], in_=h1p[:], func=FT.Tanh,
                                 bias=b1)
            st_h1[j] = h1

        def l2_emit(j):
            h2p = mm2.tile([HID, BLKN], f32, tag="h2p", name="h2p")
            nc.tensor.matmul(h2p[:], w2, st_h1.pop(j)[:], start=True,
                             stop=True)
            st_h2p[j] = h2p

        def tanh2_emit(j):
            h2 = hpool.tile([HID, BLKN], f16, tag="h2", name="h2")
            nc.scalar.activation(out=h2[:], in_=st_h2p.pop(j)[:],
                                 func=FT.Tanh, bias=b2)
            st_h2[j] = h2

        def l3_emit(j):
            qp = mmq.tile([CD, BLKN], f32, tag="qp", name="qp")
            nc.tensor.matmul(qp[:], w3, st_h2.pop(j)[:], start=True,
                             stop=True)
            st_qp[j] = qp

        def r_emit(j, rps):
            nc.tensor.matmul(rps[:], rsel[:, 32 * j:32 * j + 32],
                             st_qq.pop(j)[:], start=(j == 0),
                             stop=(j == NBLK - 1))

        def qm_emit(j):
            n0 = j * BLKN
            nc.vector.tensor_scalar(out=qsb[:, n0:n0 + BLKN],
                                    in0=st_qp.pop(j)[:], scalar1=b3,
                                    scalar2=None, op0=ALU.add)

        def qs_emit(j):
            n0 = j * BLKN
            qq = qqpool.tile([CD, BLKN], f16, tag="qq", name="qq")
            nc.gpsimd.tensor_tensor(out=qq[:], in0=qsb[:, n0:n0 + BLKN],
                                    in1=qsb[:, n0:n0 + BLKN], op=ALU.mult)
            st_qq[j] = qq

        def cb_emit(j, cstack):
            su, so = divmod(j, SUPER)
            if so == 0:
                st_ut[su] = opool.tile([CD, SUPN], f16, tag="ut", name="ut")
            cb = cps.tile([CD, BLKN], f32, tag="cb", name="cb")
            nc.tensor.matmul(cb[:], sels[:, 96 * j:96 * j + 96],
                             cstack[:], start=True, stop=True)
            st_cb[j] = cb

        def ut_emit(j):
            su, so = divmod(j, SUPER)
            n0 = j * BLKN
            ns = slice(so * BLKN, (so + 1) * BLKN)
            nc.vector.tensor_tensor(out=st_ut[su][:, ns],
                                    in0=qsb[:, n0:n0 + BLKN],
                                    in1=st_cb.pop(j)[:], op=ALU.mult)
            if so == SUPER - 1:
                nc.sync.dma_start(out=uT_d[su], in_=st_ut.pop(su)[:])

        def ph1_tick(T, rps, tsp):
            p = T
            if 0 <= p < NPAIR:
                l1_emit(2 * p, rps)
                l1_emit(2 * p + 1, rps)
                l1_tail(2 * p)
                l1_tail(2 * p + 1)
                t_emit(2 * p, tsp)
                t_emit(2 * p + 1, tsp)
            p = T - STG_T1
            if 0 <= p < NPAIR:
                tanh1_emit(2 * p)
                tanh1_emit(2 * p + 1)
            p = T - STG_L2
            if 0 <= p < NPAIR:
                l2_emit(2 * p)
                l2_emit(2 * p + 1)
            p = T - STG_T2
            if 0 <= p < NPAIR:
                tanh2_emit(2 * p)
                tanh2_emit(2 * p + 1)
            p = T - STG_L3
            pr = T - R_LAG
            if 0 <= p < NPAIR:
                l3_emit(2 * p)
            if 0 <= pr < NPAIR:
                r_emit(2 * pr, rps)
            if 0 <= p < NPAIR:
                l3_emit(2 * p + 1)
            if 0 <= pr < NPAIR:
                r_emit(2 * pr + 1, rps)
            p = T - STG_QM
            if 0 <= p < NPAIR:
                qm_emit(2 * p)
                qm_emit(2 * p + 1)
            p = T - STG_QS
            if 0 <= p < NPAIR:
                qs_emit(2 * p)
                qs_emit(2 * p + 1)

        def ph3_half(T, cstack, second):
            p = T - OFF3
            if not (0 <= p < NPAIR):
                return
            j = 2 * p + (1 if second else 0)
            cb_emit(j, cstack)
            ut_emit(j)

        prev = None      # cstack of previous iteration
        for rep in range(REPEAT):
            st_x.clear()
            rps = rpsp.tile([32, BLKN], f32, tag="rps", name="rps")
            tsp = tpool.tile([128, BLKN], f16, tag="tsp", name="tsp")
            nc.scalar.dma_start(out=tsp[:], in_=tsp_d[:])
            cstack = ppool.tile([32, BLKN], f16, tag="cst", name="cstack")
            p2jobs = None
            for T in range(NTICK):
                if prev is not None:
                    ph3_half(T, prev[1], False)
                ph1_tick(T, rps, tsp)
                if T == 1 and prev is not None:
                    # grouped emission bounds ACT table switches to 2
                    for jb in prev[0]:
                        jb()
                if prev is not None:
                    ph3_half(T, prev[1], True)
            p2jobs = phase2(rps, cstack)
            prev = (p2jobs, cstack)

        # drain the last iteration's solve + ph3
        for jb in prev[0]:
            jb()
        for T in range(NTICK):
            ph3_half(T, prev[1], False)
            ph3_half(T, prev[1], True)

    nc.compile()
    return nc


def _host_constants(W1, b1, W2, b2, W3, b3):
    S = _scatter_matrix()
    f16 = np.float16
    W1 = np.asarray(W1, np.float32)
    W1z, W1t = W1[1:, :], W1[0, :]
    w3 = (-(np.asarray(W3, np.float64) @ S)).astype(np.float32)
    b3n = (-(np.asarray(b3, np.float64) @ S)).astype(np.float32)

    cpk16 = np.zeros((128, C16), f16)
    cpk16[0:128, 0:100] = W1z[0:128].astype(f16)
    cpk16[0:64, 100:200] = W1z[128:192].astype(f16)
    cpk16[64:128, 100:200] = W1z[128:192].astype(f16)
    cpk16[:, 200:300] = W1t.astype(f16)[None, :]
    cpk16[0:100, 300:400] = np.asarray(W2, np.float32).astype(f16)
    cpk16[0:100, 400:496] = w3.astype(f16)
    cpk16[:, 496:593] = 1.0

    cpk32 = np.zeros((128, 8), np.float32)
    cpk32[:, 4] = 2.0 / 27.0
    cpk32[:, 5] = 1.0 / 27.0
    cpk32[0:100, 0] = np.asarray(b1, np.float32)
    cpk32[0:100, 1] = np.asarray(b2, np.float32)
    cpk32[0:96, 3] = b3n

    rsel = np.zeros((96, 32 * 32), f16)
    for j in range(32):
        rsel[:, 32 * j + j] = 1.0
    sels = np.zeros((32, 32 * 96), f16)
    for j in range(32):
        sels[j, 96 * j:96 * j + 96] = 1.0
    tsel = np.zeros((128, 11 * 100), f16)
    for s in range(3):
        for v in range(11):
            if 3 * v + s < NBLK:
                tsel[32 * s + v, 100 * v:100 * v + 100] = W1t.astype(f16)
    return {"cpk16": cpk16, "cpk32": cpk32, "rsel": rsel, "sels": sels,
            "tsel": tsel}


def _shard_inputs(z, t, consts):
    f16 = np.float16
    in_maps = []
    for c in range(N_CORES):
        sl = slice(c * BC, (c + 1) * BC)
        zc = np.asarray(z[sl], np.float32).astype(f16)
        tc = np.asarray(t[sl], np.float32).astype(f16).reshape(BC)
        m = dict(consts)
        xa = zc.T[0:128].reshape(128, NSUP, SUPN).transpose(1, 0, 2)
        m["xa"] = np.ascontiguousarray(xa)
        tails = zc.T[128:192].reshape(64, NBLK, BLKN)
        xb2 = np.zeros((NSUP, 128, SUPN // 2), f16)
        for b in range(NBLK):
            su, so = divmod(b, SUPER)
            mm, k = so % 2, so // 2
            xb2[su, 64 * mm:64 * mm + 64, 512 * k:512 * k + 512] = tails[:, b]
        m["xb2"] = xb2
        tsp = np.zeros((128, BLKN), f16)
        for b in range(NBLK):
            tsp[_tsp_row(b)] = tc[b * BLKN:(b + 1) * BLKN]
        m["tsp"] = tsp
        in_maps.append(m)
    return in_maps


def _unshard_output(uT):
    full = np.asarray(uT, np.float32).transpose(1, 0, 2).reshape(CD, BC)
    return np.ascontiguousarray(full.T)


def _get_program():
    key = (REPEAT,)
    if key not in _PROG_CACHE:
        _PROG_CACHE[key] = _build_program()
    return _PROG_CACHE[key]


def kernel(z, t, W1, b1, W2, b2, W3, b3, _trace=False):
    from concourse.bass_utils import run_bass_kernel_spmd

    consts = _host_constants(W1, b1, W2, b2, W3, b3)
    nc = _get_program()
    in_maps = _shard_inputs(np.asarray(z), np.asarray(t), consts)
    res = run_bass_kernel_spmd(nc, in_maps, list(range(N_CORES)),
                               trace=_trace)
    outs = [_unshard_output(res.results[c]["uT"]) for c in range(N_CORES)]
    u = np.concatenate(outs, axis=0).astype(np.float32)
    if _trace:
        return u, res
    return u


def _make_runner(in_maps):
    import jax
    import numpy as _np
    from jax.sharding import Mesh, PartitionSpec
    from jax.experimental.shard_map import shard_map
    import concourse.mybir as mybir
    from concourse import bass2jax

    nc = _get_program()
    bass2jax.install_neuronx_cc_hook()

    partition_name = (nc.partition_id_tensor.name
                      if nc.partition_id_tensor else None)
    in_names, out_names, out_avals, zero_outs = [], [], [], []
    for alloc in nc.m.functions[0].allocations:
        if not isinstance(alloc, mybir.MemoryLocationSet):
            continue
        name = alloc.memorylocations[0].name
        if alloc.kind == "ExternalInput":
            if name != partition_name:
                in_names.append(name)
        elif alloc.kind == "ExternalOutput":
            shape = list(alloc.tensor_shape)
            dt = mybir.dt.np(alloc.dtype)
            out_names.append(name)
            out_avals.append(jax.core.ShapedArray(shape, dt))
            zero_outs.append(_np.zeros(shape, dt))
    in_names_full = in_names + out_names
    if partition_name is not None:
        in_names_full.append(partition_name)

    def _body(*args):
        operands = list(args)
        if partition_name is not None:
            operands.append(bass2jax.partition_id_tensor())
        outs = bass2jax._bass_exec_p.bind(
            *operands,
            out_avals=tuple(out_avals),
            in_names=tuple(in_names_full),
            out_names=tuple(out_names),
            lowering_input_output_aliases=(),
            sim_require_finite=True,
            sim_require_nnan=True,
            nc=nc,
        )
        return tuple(outs)

    devices = jax.devices()[:N_CORES]
    mesh = Mesh(np.asarray(devices), ("core",))
    nin = len(in_names) + len(zero_outs)
    fn = jax.jit(shard_map(_body, mesh=mesh,
                           in_specs=(PartitionSpec("core"),) * nin,
                           out_specs=(PartitionSpec("core"),) * len(out_names),
                           check_rep=False), keep_unused=True)
    concat = [_np.concatenate([in_maps[c][n] for c in range(N_CORES)], axis=0)
              for n in in_names]
    concat += [_np.zeros((N_CORES * zz.shape[0], *zz.shape[1:]), zz.dtype)
               for zz in zero_outs]
    sh = jax.sharding.NamedSharding(mesh, PartitionSpec("core"))
    dev_in = [jax.device_put(a, sh) for a in concat]
    return fn, dev_in, out_names
